# revision 11
# baseline (speedup 1.0000x reference)
"""Trainium2 Bass kernel for nn_MultiMPNN (gnn_message_passing).

Reference computation (B=4, N=512, Z=64, E=16, H=128):
    msgs[b,i,j,:] = z[b,i]@W_i + z[b,j]@W_j + e_feat[b,i,j]@W_e + b_msg
    agg[b,i,:]    = max_j (msgs + (adj>0 ? 0 : -inf))
    out           = z@Wu_z + agg@Wu_h + b_upd

Sharding: 8 cores = (batch b, half of destination rows i).  Each core owns
256 i-rows and the full j axis.

Device-side structure (v2):
 1. Per i-row, ONE fp8 matmul with augmented contraction K = E + Z = 80:
      lhsT_aug[80,128] = [32*W_e ; 32*W_j]                  (constant, e3m4)
      rhs_aug [80,w]   = [2*e_feat[b,i,sel].T ; 2*z[b,sel].T] (streamed, e3m4)
      PSUM[h,j] = 64*(ze + zj)  ->  max over j -> agg column
    zi + b_msg commute out of the max; they are folded into the final
    linear on the host (zit pre-scaled by 64, Wu_h divided by 64).
 2. The host compacts the j axis per row (only adj=1 columns participate);
    pad columns REPLICATE the row's first active column, which leaves the
    max unchanged and removes the need for a -inf mask plane entirely
    (so K=80 fits e3m4's tiny range and the stream has no mask row).
 3. e3m4 (1 byte) halves DMA vs bf16; with the x2/x32 scaling all values
    sit in e3m4's normal range (max 15.5, min normal 0.25).  Measured
    end-to-end rel err ~6e-3 vs the 2e-2 gate.
 4. The PSUM drain is split across three engine paths to run all engines
    in parallel (the drain, not the matmul, is the bottleneck):
      D: DVE reduce_max straight from PSUM        [H,4,w] -> [H,4]
      T: ACT copies PSUM->SBUF bf16, then one DVE tensor_tensor_reduce
         per row (halves as in0/in1, fused elementwise-max + reduce)
      G: ACT copies PSUM->SBUF bf16, then GPSIMD tensor_reduce
"""

import numpy as np
import ml_dtypes

import concourse.bacc as bacc
import concourse.mybir as mybir
import concourse.tile as tile
from concourse import bass_utils
from concourse.bass_interp import get_hw_module
from contextlib import ExitStack

B, N, Z, E, H = 4, 512, 64, 16, 128
NCORES = 8
IH = N * B // NCORES          # 256 destination rows per core
KAUG = E + Z                  # 80 (no mask plane; pads replicate a real col)
RG = 4                        # rows per PSUM tile / drain group
BANK = 512                    # f32 elems per PSUM bank
SCALE_X = 2.0                 # host scale on streamed data (e, z)
SCALE_W = 32.0                # host scale on stationary weights
# combined matmul output scale = SCALE_X*SCALE_W = 64; undone in the final
# linear (wuh/64) and the zit term (x64).

# Drain schedule: rotating pattern of group types.
#   'D' = DVE reduce_max direct from PSUM (DVE-only, no stage)
#   'T' = ACT stage -> per-row DVE tensor_tensor_reduce
# (GPSIMD tensor ops are rejected by this compiler build: generic
#  TensorTensor fails the Pool-engine ISA check, so no third lane.)
PATTERN = ['T', 'T', 'T', 'D', 'T', 'T', 'T', 'D',
           'T', 'T', 'D', 'T', 'T', 'T', 'D', 'T']
# 'T' drain implementation: "ttr" = per-row tensor_tensor_reduce (faster,
# but the ISA op may not run on all HW); "tree" = TT-max tree + reduce
# (baseline-proven).
TTR_MODE = "tree"
# Dummy keep-alive matmuls per group (keeps the PE HAM-warm when drains
# pace the pipeline).  0 disables.
DUMMIES_EVERY = 0

F32 = mybir.dt.float32
BF16 = mybir.dt.bfloat16
FP8 = mybir.dt.float8e3
NP_BF16 = ml_dtypes.bfloat16
NP_FP8 = ml_dtypes.float8_e3m4
FP8_MAX = 15.5

TRACE = False                 # test.py sets True to capture an NTFF profile
TRACE_DIR = None              # optional fixed dir for trace artifacts
LAST_RESULTS = None           # BassKernelResults of the last run (for test.py)

_MODULE_CACHE = {}


def _ensure_ntff_hook():
    """The agent image's antenv lacks axon_hooks; recreate it so
    run_bass_kernel_spmd(trace=True) can reach the axon NTFF profiler."""
    import sys
    import types

    try:
        import antenv.axon_hooks  # noqa: F401

        return
    except ImportError:
        pass
    import antenv
    from trn_agent_boot.trn_boot import _ntff_profile_via_ctypes

    state = {"h": _ntff_profile_via_ctypes("/opt/axon/libaxon_pjrt.so")}
    mod = types.ModuleType("antenv.axon_hooks")
    mod.get_axon_ntff_profile_hook = lambda: state["h"]
    mod.set_axon_ntff_profile_hook = lambda h: state.__setitem__("h", h)
    sys.modules["antenv.axon_hooks"] = mod
    antenv.axon_hooks = mod


def _build_module(widths):
    widths = list(widths)                    # one width per RG-row group
    row_w = [w for w in widths for _ in range(RG)]
    offs = [0]
    for w in row_w:
        offs.append(offs[-1] + w)
    tot = offs[-1]
    nc = bacc.Bacc(
        "TRN2",
        target_bir_lowering=False,
        debug=False,
        enable_asserts=False,
        num_devices=NCORES,
    )

    stream = nc.dram_tensor("stream", [KAUG, tot], FP8, kind="ExternalInput")
    lhst = nc.dram_tensor("lhst", [KAUG, H], FP8, kind="ExternalInput")
    zit = nc.dram_tensor("zit", [H, IH], F32, kind="ExternalInput")
    hostc = nc.dram_tensor("hostc", [H, IH], F32, kind="ExternalInput")
    wuh = nc.dram_tensor("wuh", [H, H], F32, kind="ExternalInput")
    ident = nc.dram_tensor("ident", [H, H], F32, kind="ExternalInput")
    out = nc.dram_tensor("out", [IH, H], F32, kind="ExternalOutput")

    with ExitStack() as ctx:
        tc = ctx.enter_context(tile.TileContext(nc))
        const = ctx.enter_context(tc.tile_pool(name="const", bufs=1))
        mega = ctx.enter_context(tc.tile_pool(name="mega", bufs=4))
        stage_pool = ctx.enter_context(tc.tile_pool(name="stage", bufs=6))
        scr_pool = ctx.enter_context(tc.tile_pool(name="scr", bufs=10))
        psum = ctx.enter_context(tc.tile_pool(name="psum", bufs=2, space="PSUM"))

        lhst_sb = const.tile([KAUG, H], FP8, tag="lhst")
        nc.scalar.dma_start(lhst_sb[:, :], lhst.ap())
        zit_sb = const.tile([H, IH], F32, tag="zit")
        nc.scalar.dma_start(zit_sb[:, :], zit.ap())
        hostc_sb = const.tile([H, IH], F32, tag="hostc")
        nc.scalar.dma_start(hostc_sb[:, :], hostc.ap())
        wuh_sb = const.tile([H, H], F32, tag="wuh")
        nc.scalar.dma_start(wuh_sb[:, :], wuh.ap())
        ident_sb = const.tile([H, H], F32, tag="ident")
        nc.scalar.dma_start(ident_sb[:, :], ident.ap())

        magg = const.tile([H, IH], F32, tag="magg")

        # PE warm-up (HAM clock gate: sustained activity releases the 4/8
        # throttle) + engine warm-ups: first ACT table load and first
        # GPSIMD op overlap the DMA-dominated startup window.
        warm_rhs = const.tile([KAUG, BANK], FP8, tag="warm_rhs")
        nc.vector.memset(warm_rhs[:, :], 0.0)
        warm_bf = const.tile([H, 64], BF16, tag="warm_bf")
        nc.vector.memset(warm_bf[:, :], 0.0)
        nc.scalar.copy(warm_bf[:, :64], warm_bf[:, :64])
        pw = psum.tile([H, RG * BANK], F32, tag="ps")
        for _ in range(8):
            nc.tensor.matmul(
                pw[:, :BANK], lhst_sb[:, :], warm_rhs[:, :], start=True, stop=True
            )

        # Ramp-up: small first blocks so the PE starts within ~1 us of
        # launch; bigger late blocks for DMA packet efficiency.
        sizes = [4, 4, 8, 16] + [32] * 7
        assert sum(sizes) == IH

        stream_ap = stream.ap()
        row0 = 0
        gcount = 0
        for blk, gsz in enumerate(sizes):
            belems = offs[row0 + gsz] - offs[row0]
            mb = mega.tile([KAUG, belems], FP8, tag="mega")
            nc.sync.dma_start(
                mb[:, :],
                stream_ap[:, offs[row0] : offs[row0 + gsz]],
            )
            ngrp = gsz // RG
            for g4 in range(ngrp):
                gidx = row0 // RG + g4
                w = widths[gidx]
                hw = w // 2
                i0 = row0 + g4 * RG
                gtype = PATTERN[gcount % len(PATTERN)]
                gcount += 1

                ps = psum.tile([H, RG * BANK], F32, tag="ps")
                if DUMMIES_EVERY and (gcount % DUMMIES_EVERY == 0) and blk >= 2:
                    nc.tensor.matmul(
                        ps[:, :BANK], lhst_sb[:, :], warm_rhs[:, :],
                        start=True, stop=True,
                    )
                for r in range(RG):
                    o = offs[i0 + r] - offs[row0]
                    nc.tensor.matmul(
                        ps[:, r * BANK : r * BANK + w],
                        lhst_sb[:, :],
                        mb[:, o : o + w],
                        start=True,
                        stop=True,
                    )
                ps_rows = ps[:, :].rearrange("p (g j) -> p g j", g=RG)

                if gtype == 'D':
                    nc.vector.reduce_max(
                        magg[:, i0 : i0 + RG],
                        ps_rows[:, :, :w],
                        axis=mybir.AxisListType.X,
                    )
                else:
                    stage = stage_pool.tile([H, RG * w], BF16, tag="stage")
                    st_rows = stage[:, :].rearrange("p (g j) -> p g j", g=RG)
                    nc.scalar.copy(st_rows[:, :, :], ps_rows[:, :, :w])
                    if TTR_MODE == "ttr":
                        for r in range(RG):
                            scr = scr_pool.tile([H, hw], BF16, tag="scr")
                            nc.vector.tensor_tensor_reduce(
                                out=scr[:, :],
                                in0=stage[:, r * w : r * w + hw],
                                in1=stage[:, r * w + hw : (r + 1) * w],
                                scale=1.0,
                                scalar=-1.0e30,
                                op0=mybir.AluOpType.max,
                                op1=mybir.AluOpType.max,
                                accum_out=magg[:, i0 + r : i0 + r + 1],
                            )
                    else:
                        qw = w // 4
                        half = scr_pool.tile([H, RG * hw], BF16, tag="half")
                        hf_rows = half[:, :].rearrange("p (g j) -> p g j", g=RG)
                        nc.vector.tensor_tensor(
                            hf_rows[:, :, :],
                            st_rows[:, :, :hw],
                            st_rows[:, :, hw:],
                            mybir.AluOpType.max,
                        )
                        quar = scr_pool.tile([H, RG * qw], BF16, tag="quar")
                        qr_rows = quar[:, :].rearrange("p (g j) -> p g j", g=RG)
                        nc.vector.tensor_tensor(
                            qr_rows[:, :, :],
                            hf_rows[:, :, :qw],
                            hf_rows[:, :, qw:],
                            mybir.AluOpType.max,
                        )
                        nc.vector.reduce_max(
                            magg[:, i0 : i0 + RG],
                            qr_rows[:, :, :],
                            axis=mybir.AxisListType.X,
                        )
            row0 += gsz

        aggt = const.tile([H, IH], F32, tag="aggt")
        nc.vector.tensor_add(aggt[:, :], magg[:, :], zit_sb[:, :])

        psf = psum.tile([H, RG * BANK], F32, tag="ps")
        nc.tensor.matmul(psf[:, :IH], wuh_sb[:, :], aggt[:, :], start=True, stop=True)

        outt = const.tile([H, IH], F32, tag="outt")
        nc.vector.tensor_add(outt[:, :], psf[:, :IH], hostc_sb[:, :])

        out_ap = out.ap()
        for t in range(IH // H):
            pst = psum.tile([H, RG * BANK], F32, tag="ps")
            nc.tensor.transpose(
                pst[:, :H], outt[:, t * H : (t + 1) * H], ident_sb[:, :]
            )
            osb = const.tile([H, H], F32, tag=f"osb{t}")
            nc.scalar.copy(osb[:, :], pst[:, :H])
            nc.sync.dma_start(out_ap[t * H : (t + 1) * H, :], osb[:, :])

    nc.compile()
    nc.m = get_hw_module(nc.m)
    return nc


def _fp8(x):
    return np.asarray(
        np.clip(x, -FP8_MAX, FP8_MAX), dtype=NP_FP8
    )


def _prepare(z, e_feat, adj, W_msg, b_msg, W_upd, b_upd):
    """Host-side sharding + compaction with per-group widths.

    Rows are sorted by active-edge count (descending) so each group of RG
    rows gets a tight shared width.  Pad columns replicate the row's first
    active column (max-neutral), so no mask plane is needed.  Returns
    (in_maps, widths, orders); out rows come back permuted by `orders`.
    """
    W_i, W_j, W_e = W_msg[:Z], W_msg[Z : 2 * Z], W_msg[2 * Z :]
    Wu_z, Wu_h = W_upd[:Z], W_upd[Z:]

    counts = (adj > 0).sum(axis=-1)                   # [B, N]
    orders, csort = [], []
    for c in range(NCORES):
        b, half = divmod(c, NCORES // B)
        cnt = counts[b, half * IH : (half + 1) * IH]
        order = np.argsort(-cnt, kind="stable")
        orders.append(order)
        csort.append(cnt[order])
    csort = np.stack(csort)                           # [NCORES, IH]
    gmax = csort.reshape(NCORES, IH // RG, RG).max(-1).max(0)
    widths = np.clip((gmax + 7) // 8 * 8, 16, N).astype(int)  # [IH//RG]
    row_w = np.repeat(widths, RG)
    offs = np.concatenate([[0], np.cumsum(row_w)])
    tot = int(offs[-1])
    maxw = int(widths.max())

    lhst_np = _fp8(np.concatenate([W_e, W_j], axis=0) * SCALE_W)  # [80, H]
    wuh_np = np.ascontiguousarray(Wu_h / (SCALE_X * SCALE_W), np.float32)
    ident_np = np.eye(H, dtype=np.float32)

    in_maps = []
    for c in range(NCORES):
        b, half = divmod(c, NCORES // B)
        sl = slice(half * IH, (half + 1) * IH)
        order = orders[c]
        adj_blk = (adj[b, sl] > 0)[order]             # [IH, N] sorted rows
        cnt = adj_blk.sum(-1)                          # [IH]
        jorder = np.argsort(~adj_blk, axis=-1, kind="stable")[:, :maxw]
        # pad columns replicate the first (active) column
        ar = np.arange(maxw)[None, :]
        jsel = np.where(ar < cnt[:, None], jorder, jorder[:, :1])
        e_sel = np.take_along_axis(
            e_feat[b, sl][order], jsel[:, :, None], axis=1
        )                                             # [IH, maxw, E]
        z_sel = z[b][jsel]                            # [IH, maxw, Z]

        stream = np.empty((KAUG, tot), dtype=NP_FP8)
        for r in range(IH):
            w = row_w[r]
            o = offs[r]
            stream[:E, o : o + w] = _fp8(e_sel[r, :w].T * SCALE_X)
            stream[E:, o : o + w] = _fp8(z_sel[r, :w].T * SCALE_X)

        zperm = z[b, sl][order]
        in_maps.append(
            {
                "stream": stream,
                "lhst": lhst_np,
                "zit": np.ascontiguousarray(
                    ((zperm @ W_i).T + b_msg[:, None]) * (SCALE_X * SCALE_W),
                    dtype=np.float32,
                ),
                "hostc": np.ascontiguousarray(
                    (zperm @ Wu_z + b_upd).T, dtype=np.float32
                ),
                "wuh": wuh_np,
                "ident": ident_np,
            }
        )
    return in_maps, widths, orders


def kernel(z, e_feat, adj, W_msg, b_msg, W_upd, b_upd):
    global LAST_RESULTS

    z = np.asarray(z, np.float32)
    e_feat = np.asarray(e_feat, np.float32)
    adj = np.asarray(adj)
    W_msg = np.asarray(W_msg, np.float32)
    b_msg = np.asarray(b_msg, np.float32)
    W_upd = np.asarray(W_upd, np.float32)
    b_upd = np.asarray(b_upd, np.float32)

    in_maps, widths, orders = _prepare(z, e_feat, adj, W_msg, b_msg, W_upd, b_upd)

    key = tuple(widths)
    if key not in _MODULE_CACHE:
        _MODULE_CACHE[key] = _build_module(widths)
    nc = _MODULE_CACHE[key]

    if TRACE:
        _ensure_ntff_hook()
    res = bass_utils.run_bass_kernel_spmd(
        nc, in_maps, core_ids=list(range(NCORES)), trace=TRACE, tmpdir=TRACE_DIR
    )
    LAST_RESULTS = res

    full = np.empty((B, N, H), np.float32)
    for c in range(NCORES):
        b, half = divmod(c, NCORES // B)
        full[b, half * IH + orders[c]] = res.results[c]["out"]
    return full


if __name__ == "__main__":
    rng = np.random.default_rng(0)
    ins = {
        "z": rng.standard_normal((B, N, Z)).astype(np.float32),
        "e_feat": rng.standard_normal((B, N, N, E)).astype(np.float32),
        "adj": (rng.random((B, N, N)) < 0.5).astype(np.int32),
        "W_msg": (rng.standard_normal((2 * Z + E, H)) * 0.1).astype(np.float32),
        "b_msg": np.zeros(H, np.float32),
        "W_upd": (rng.standard_normal((Z + H, H)) * 0.1).astype(np.float32),
        "b_upd": np.zeros(H, np.float32),
    }
    out = kernel(**ins)
    print("out", out.shape, out.dtype, float(np.abs(out).max()))
